# revision 1
# baseline (speedup 1.0000x reference)
"""GQA attention kernel for Trainium2, 8-core tensor-parallel (by heads).

Shapes (hardcoded from the problem spec):
  x:(4,128,4096) fp32, wq:(4096,4096), wk/wv:(4096,1024), wo:(4096,4096),
  32 q heads / 8 kv heads, head_dim 128, start_pos=0 (cache is overwritten).

Sharding: core c owns q heads [4c,4c+4) and kv head c; wq/wk/wv column-
sharded, wo row-sharded; each core computes a full (512,4096) partial of
the output projection; host sums the 8 partials and adds bo.
"""
import sys
sys.path.insert(0, "/opt/trn_rl_repo")

import numpy as np

B, S, D = 4, 128, 4096
H, KV, HD = 32, 8, 128
NCORES = 8
HQ = H // NCORES          # 4 q heads per core
T = B * S                 # 512 tokens
FQ = HQ * HD              # 512 q features per core
SCALE = 1.0 / float(np.sqrt(HD))

_CACHE = {}


def _build():
    import concourse.bass as bass
    import concourse.tile as tile
    from concourse import bacc, mybir

    F32, F32R = mybir.dt.float32, mybir.dt.float32r
    AF = mybir.ActivationFunctionType

    nc = bacc.Bacc("TRN2", target_bir_lowering=False, debug=False,
                   enable_asserts=False, num_devices=NCORES)

    xT_d = nc.dram_tensor("xT", [D, T], F32R, kind="ExternalInput").ap()
    wq_d = nc.dram_tensor("wq", [D, FQ], F32R, kind="ExternalInput").ap()
    wkv_d = nc.dram_tensor("wkv", [D, 2 * HD], F32R, kind="ExternalInput").ap()
    wo_d = nc.dram_tensor("wo", [FQ, D], F32R, kind="ExternalInput").ap()
    bq_d = nc.dram_tensor("bq", [1, FQ], F32, kind="ExternalInput").ap()
    bkv_d = nc.dram_tensor("bkv", [1, 2 * HD], F32, kind="ExternalInput").ap()
    c4_d = nc.dram_tensor("c4", [S, HQ * 64], F32, kind="ExternalInput").ap()
    s4_d = nc.dram_tensor("s4", [S, HQ * 64], F32, kind="ExternalInput").ap()
    mk_d = nc.dram_tensor("mk", [S, HQ * S], F32, kind="ExternalInput").ap()
    on_d = nc.dram_tensor("on", [S, S], F32, kind="ExternalInput").ap()
    id_d = nc.dram_tensor("idm", [S, S], F32, kind="ExternalInput").ap()
    out_d = nc.dram_tensor("out", [T, D], F32, kind="ExternalOutput").ap()

    NK = D // 128   # 32 contraction chunks

    with tile.TileContext(nc) as tc:
        with tc.tile_pool(name="consts", bufs=1) as cp:
            b128q = cp.tile([128, FQ], F32)
            b128kv = cp.tile([128, 2 * HD], F32)
            c4 = cp.tile([128, HQ * 64], F32)
            s4 = cp.tile([128, HQ * 64], F32)
            mk = cp.tile([128, HQ * S], F32)
            ones = cp.tile([128, S], F32)
            ident = cp.tile([128, S], F32)
            nc.gpsimd.dma_start(b128q, bass.AP(tensor=bq_d.tensor, offset=0,
                                               ap=[[0, 128], bq_d.ap[1]]))
            nc.gpsimd.dma_start(b128kv, bass.AP(tensor=bkv_d.tensor, offset=0,
                                                ap=[[0, 128], bkv_d.ap[1]]))
            nc.sync.dma_start(c4, c4_d)
            nc.sync.dma_start(s4, s4_d)
            nc.sync.dma_start(mk, mk_d)
            nc.sync.dma_start(ones, on_d)
            nc.sync.dma_start(ident, id_d)

            with tc.tile_pool(name="qkvs", bufs=4) as qp, \
                 tc.tile_pool(name="ropep", bufs=4) as rp, \
                 tc.tile_pool(name="tmpp", bufs=2) as tp, \
                 tc.tile_pool(name="trs", bufs=4) as trp, \
                 tc.tile_pool(name="attn", bufs=2) as ap_, \
                 tc.tile_pool(name="aop", bufs=4) as aop, \
                 tc.tile_pool(name="outp", bufs=8) as op:

                # ---------------- Phase A: QKV projections -------------
                q_sb = [None] * B
                kv_sb = [None] * B
                with tc.tile_pool(name="psA", bufs=4, space="PSUM") as psA, \
                     tc.tile_pool(name="xtp", bufs=10) as xp, \
                     tc.tile_pool(name="wp", bufs=10) as wp:
                    pq = [psA.tile([128, FQ], F32, tag="pq", name=f"pq{m}")
                          for m in range(B)]
                    pkv = [psA.tile([128, 2 * HD], F32, tag="pkv", name=f"pkv{m}")
                           for m in range(B)]
                    for k in range(NK):
                        xt = xp.tile([128, T], F32R, tag="xt", name=f"xt{k}")
                        (nc.sync if k % 2 == 0 else nc.scalar).dma_start(xt, xT_d[k * 128:(k + 1) * 128, :])
                        wqt = wp.tile([128, FQ], F32R, tag="wqt", name=f"wqt{k}")
                        (nc.scalar if k % 2 == 0 else nc.sync).dma_start(wqt, wq_d[k * 128:(k + 1) * 128, :])
                        wkvt = wp.tile([128, 2 * HD], F32R, tag="wkvt", name=f"wkvt{k}")
                        nc.scalar.dma_start(wkvt, wkv_d[k * 128:(k + 1) * 128, :])
                        for m in range(B):
                            lhs = xt[:, m * 128:(m + 1) * 128]
                            nc.tensor.matmul(pq[m], lhs, wqt,
                                             start=(k == 0), stop=(k == NK - 1))
                            nc.tensor.matmul(pkv[m], lhs, wkvt,
                                             start=(k == 0), stop=(k == NK - 1))
                    for m in range(B):
                        q_sb[m] = qp.tile([128, FQ], F32, tag="q", name=f"q{m}")
                        nc.vector.tensor_add(q_sb[m], pq[m], b128q)
                        kv_sb[m] = qp.tile([128, 2 * HD], F32, tag="kv", name=f"kv{m}")
                        nc.vector.tensor_add(kv_sb[m], pkv[m], b128kv)

                # ------------- Phases B-D per batch tile ---------------
                with tc.tile_pool(name="psB", bufs=1, space="PSUM") as psB, \
                     tc.tile_pool(name="wop", bufs=16) as wop:
                    c4v = c4.rearrange("p (h r) -> p h r", h=HQ)
                    s4v = s4.rearrange("p (h r) -> p h r", h=HQ)
                    aoT = [None] * B
                    for m in range(B):
                        # RoPE on q
                        qv = q_sb[m].rearrange("p (h r two) -> p h r two",
                                               h=HQ, r=64, two=2)
                        q_e, q_o = qv[:, :, :, 0], qv[:, :, :, 1]
                        qr = rp.tile([128, FQ], F32, tag="qr", name=f"qr{m}")
                        qrv = qr.rearrange("p (h r two) -> p h r two",
                                           h=HQ, r=64, two=2)
                        t1 = tp.tile([128, HQ * 64], F32, tag="t1", name=f"t1_{m}")
                        t2 = tp.tile([128, HQ * 64], F32, tag="t2", name=f"t2_{m}")
                        t1v = t1.rearrange("p (h r) -> p h r", h=HQ)
                        t2v = t2.rearrange("p (h r) -> p h r", h=HQ)
                        nc.vector.tensor_mul(t1v, q_o, s4v)
                        nc.vector.tensor_mul(t2v, q_e, c4v)
                        nc.vector.tensor_sub(qrv[:, :, :, 0], t2v, t1v)
                        nc.vector.tensor_mul(t1v, q_o, c4v)
                        nc.vector.tensor_mul(t2v, q_e, s4v)
                        nc.vector.tensor_add(qrv[:, :, :, 1], t2v, t1v)
                        # RoPE on k (head 0 of kv tile)
                        kv_ = kv_sb[m][:, 0:HD].rearrange("p (r two) -> p r two",
                                                          r=64, two=2)
                        k_e, k_o = kv_[:, :, 0], kv_[:, :, 1]
                        kr = rp.tile([128, HD], F32, tag="kr", name=f"kr{m}")
                        krv = kr.rearrange("p (r two) -> p r two", r=64, two=2)
                        t1k = t1v[:, 0, :]
                        t2k = t2v[:, 0, :]
                        c1 = c4v[:, 0, :]
                        s1 = s4v[:, 0, :]
                        nc.vector.tensor_mul(t1k, k_o, s1)
                        nc.vector.tensor_mul(t2k, k_e, c1)
                        nc.vector.tensor_sub(krv[:, :, 0], t2k, t1k)
                        nc.vector.tensor_mul(t1k, k_o, c1)
                        nc.vector.tensor_mul(t2k, k_e, s1)
                        nc.vector.tensor_add(krv[:, :, 1], t2k, t1k)

                        # Transposes -> qT [d,(h,i)], kT [d,j]
                        pstq = psB.tile([128, FQ], F32, tag="pstq", name=f"pstq{m}")
                        for h in range(HQ):
                            nc.tensor.transpose(pstq[:, h * 128:(h + 1) * 128],
                                                qr[:, h * 128:(h + 1) * 128], ident)
                        qT = trp.tile([128, FQ], F32R, tag="qT", name=f"qT{m}")
                        nc.vector.tensor_copy(qT, pstq)
                        pstk = psB.tile([128, HD], F32, tag="pstk", name=f"pstk{m}")
                        nc.tensor.transpose(pstk, kr, ident)
                        kT = trp.tile([128, HD], F32R, tag="kT", name=f"kT{m}")
                        nc.scalar.copy(kT, pstk)

                        # Attention (scoresT layout [j,(h,i)])
                        psc = psB.tile([128, FQ], F32, tag="psc", bufs=2, name=f"psc{m}")
                        nc.tensor.matmul(psc, kT, qT, start=True, stop=True)
                        expT = ap_.tile([128, FQ], F32, tag="expT", name=f"expT{m}")
                        nc.scalar.activation(expT, psc, AF.Exp, scale=SCALE)
                        attn_u = ap_.tile([128, FQ], F32, tag="attn_u", name=f"au{m}")
                        nc.vector.tensor_mul(attn_u, expT, mk)
                        pden = psB.tile([128, FQ], F32, tag="pden", name=f"pden{m}")
                        nc.tensor.matmul(pden, ones, attn_u, start=True, stop=True)
                        rec = ap_.tile([128, FQ], F32, tag="rec", name=f"rec{m}")
                        nc.vector.reciprocal(rec, pden)
                        attn_n = ap_.tile([128, FQ], F32, tag="attn_n", name=f"an{m}")
                        nc.vector.tensor_mul(attn_n, attn_u, rec)
                        poT = psB.tile([128, FQ], F32, tag="poT", name=f"poT{m}")
                        nc.tensor.matmul(poT, kv_sb[m][:, HD:2 * HD], attn_n,
                                         start=True, stop=True)
                        aoT[m] = aop.tile([128, FQ], F32R, tag="aoT", name=f"aoT{m}")
                        nc.vector.tensor_copy(aoT[m], poT)

                    # ---------------- Phase D: output projection ------------
                    NT = D // 512  # 8 column tiles
                    for n in range(NT):
                        wts = []
                        for h in range(HQ):
                            wt = wop.tile([128, 512], F32R, tag="wo", name=f"wo{n}_{h}")
                            nc.scalar.dma_start(
                                wt, wo_d[h * 128:(h + 1) * 128,
                                         n * 512:(n + 1) * 512])
                            wts.append(wt)
                        for m in range(B):
                            pso = psB.tile([128, 512], F32, tag="pso", bufs=2,
                                           name=f"pso{n}_{m}")
                            for h in range(HQ):
                                nc.tensor.matmul(pso, aoT[m][:, h * 128:(h + 1) * 128],
                                                 wts[h], start=(h == 0),
                                                 stop=(h == HQ - 1))
                            osb = op.tile([128, 512], F32, tag="osb",
                                          name=f"osb{n}_{m}")
                            if (n * B + m) % 2 == 0:
                                nc.vector.tensor_copy(osb, pso)
                            else:
                                nc.scalar.copy(osb, pso)
                            nc.sync.dma_start(
                                out_d[m * 128:(m + 1) * 128,
                                      n * 512:(n + 1) * 512], osb)

    nc.compile()
    return nc


def _prep_inputs(x, freqs_cos, freqs_sin, wq, bq, wk, bk, wv, bv, wo):
    xT = np.ascontiguousarray(x.reshape(T, D).T.astype(np.float32))
    c4 = np.ascontiguousarray(np.tile(freqs_cos.astype(np.float32), (1, HQ)))
    s4 = np.ascontiguousarray(np.tile(freqs_sin.astype(np.float32), (1, HQ)))
    mk = np.ascontiguousarray(
        np.tile(np.triu(np.ones((S, S), np.float32)), (1, HQ)))
    on = np.ones((S, S), np.float32)
    idm = np.eye(S, dtype=np.float32)
    maps = []
    for c in range(NCORES):
        qs = slice(c * FQ, (c + 1) * FQ)
        ks = slice(c * HD, (c + 1) * HD)
        maps.append({
            "xT": xT,
            "wq": np.ascontiguousarray(wq[:, qs].astype(np.float32)),
            "wkv": np.ascontiguousarray(
                np.concatenate([wk[:, ks], wv[:, ks]], axis=1).astype(np.float32)),
            "wo": np.ascontiguousarray(wo[qs, :].astype(np.float32)),
            "bq": np.ascontiguousarray(bq[qs].astype(np.float32)).reshape(1, FQ),
            "bkv": np.ascontiguousarray(
                np.concatenate([bk[ks], bv[ks]]).astype(np.float32)).reshape(1, 2 * HD),
            "c4": c4, "s4": s4, "mk": mk, "on": on, "idm": idm,
        })
    return maps


def kernel(x, start_pos, freqs_cos, freqs_sin, mask, cache_k, cache_v,
           wq, bq, wk, bk, wv, bv, wo, bo, _want_trace=False):
    from concourse.bass_utils import run_bass_kernel_spmd

    assert int(start_pos) == 0
    if "nc" not in _CACHE:
        _CACHE["nc"] = _build()
    nc = _CACHE["nc"]
    in_maps = _prep_inputs(np.asarray(x), np.asarray(freqs_cos),
                           np.asarray(freqs_sin), np.asarray(wq),
                           np.asarray(bq), np.asarray(wk), np.asarray(bk),
                           np.asarray(wv), np.asarray(bv), np.asarray(wo))
    res = run_bass_kernel_spmd(nc, in_maps, core_ids=list(range(NCORES)),
                               trace=_want_trace)
    acc = np.zeros((T, D), np.float64)
    for r in res.results:
        acc += r["out"].astype(np.float64)
    out = (acc + np.asarray(bo).astype(np.float64)).astype(np.float32)
    if _want_trace:
        _CACHE["last_exec_time_ns"] = res.exec_time_ns
        _CACHE["last_trace"] = res.instructions_and_trace
    return out.reshape(B, S, D)



# revision 8
# speedup vs baseline: 71.2813x; 71.2813x over previous
"""GQA attention kernel for Trainium2, 8-core tensor-parallel (by heads).

Shapes (hardcoded from the problem spec):
  x:(4,128,4096) fp32, wq:(4096,4096), wk/wv:(4096,1024), wo:(4096,4096),
  32 q heads / 8 kv heads, head_dim 128, start_pos=0 (cache is overwritten).

Sharding: core c owns q heads [4c,4c+4) and kv head c; wq/wk/wv column-
sharded, wo row-sharded; each core computes a full (512,4096) partial of
the output projection; host sums the 8 partials and adds bo.

Device-side design (bf16 matmul path, fp32 accumulation):
  - Q/K projections computed feature-major (weights stationary, tokens
    moving) so attention needs no transposes.  Q/K features are permuted
    on the host to [evens, odds] within each head so RoPE is 6 DVE ops
    on contiguous partition halves.
  - Biases folded in as a 33rd contraction chunk (x row of ones, bias
    row in the weight pack).
  - All inputs repacked on the host into [128, k*cols] slabs so each
    array loads with O(1) large DMAs.
  - Softmax denominator via ones-matmul; 1/denom folded into the AV
    epilogue multiply.
"""
import sys
sys.path.insert(0, "/opt/trn_rl_repo")

import numpy as np
from ml_dtypes import bfloat16

B, S, D = 4, 128, 4096
H, KV, HD = 32, 8, 128
NCORES = 8
HQ = H // NCORES          # 4 q heads per core
T = B * S                 # 512 tokens
NK = D // 128 + 1         # 32 k-chunks + 1 bias chunk
QF = HQ * HD              # 512 q features per core
WQK = QF + HD             # 640 = q features + k features
SCALE = 1.0 / float(np.sqrt(HD))
PIECES = (9, 8, 8, 8)     # k-chunks per phase-A DMA piece

_CACHE = {}


def _build():
    import concourse.tile as tile
    from concourse import bacc, mybir

    F32, BF16 = mybir.dt.float32, mybir.dt.bfloat16
    AF = mybir.ActivationFunctionType

    nc = bacc.Bacc("TRN2", target_bir_lowering=False, debug=False,
                   enable_asserts=False, num_devices=NCORES)

    xt_d = nc.dram_tensor("xt", [128, NK * T], BF16, kind="ExternalInput").ap()
    wqk_d = nc.dram_tensor("wqk", [128, NK * WQK], BF16, kind="ExternalInput").ap()
    wv_d = nc.dram_tensor("wv", [128, NK * HD], BF16, kind="ExternalInput").ap()
    wo_d = nc.dram_tensor("wo", [128, HQ * D], BF16, kind="ExternalInput").ap()
    cosT_d = nc.dram_tensor("cosT", [64, T], F32, kind="ExternalInput").ap()
    sinT_d = nc.dram_tensor("sinT", [64, T], F32, kind="ExternalInput").ap()
    mkT_d = nc.dram_tensor("mkT", [128, T], BF16, kind="ExternalInput").ap()
    on_d = nc.dram_tensor("on", [128, S], BF16, kind="ExternalInput").ap()
    out_d = nc.dram_tensor("out", [T, D], F32, kind="ExternalOutput").ap()

    P0 = [0]
    for p in PIECES:
        P0.append(P0[-1] + p)

    with tile.TileContext(nc) as tc:
        with tc.tile_pool(name="w", bufs=1) as wp, \
             tc.tile_pool(name="consts", bufs=1) as cp, \
             tc.tile_pool(name="qk", bufs=1) as qkp, \
             tc.tile_pool(name="rt", bufs=4) as rtp, \
             tc.tile_pool(name="attn", bufs=2) as ap_, \
             tc.tile_pool(name="aop", bufs=1) as aop, \
             tc.tile_pool(name="outp", bufs=2) as op:

            # ---- input DMAs: phase-A slabs first, split across both rings
            xt_p, wqk_p = [], []
            for i, npc in enumerate(PIECES):
                xt_t = wp.tile([128, npc * T], BF16, name=f"xtp{i}")
                wq_t = wp.tile([128, npc * WQK], BF16, name=f"wqkp{i}")
                e1, e2 = (nc.sync, nc.scalar) if i % 2 == 0 else (nc.scalar, nc.sync)
                e1.dma_start(xt_t, xt_d[:, P0[i] * T:P0[i + 1] * T])
                e2.dma_start(wq_t, wqk_d[:, P0[i] * WQK:P0[i + 1] * WQK])
                xt_p.append(xt_t)
                wqk_p.append(wq_t)
            wv_t = wp.tile([128, NK * HD], BF16, name="wv")
            nc.sync.dma_start(wv_t, wv_d)
            cosT = cp.tile([64, T], F32)
            nc.scalar.dma_start(cosT, cosT_d)
            sinT = cp.tile([64, T], F32)
            nc.scalar.dma_start(sinT, sinT_d)
            mkT = cp.tile([128, T], BF16)
            nc.sync.dma_start(mkT, mkT_d)
            on128 = cp.tile([128, S], BF16)
            nc.scalar.dma_start(on128, on_d)
            wo_p = []
            for h in range(HQ):
                wo_t = wp.tile([128, D], BF16, name=f"wop{h}")
                (nc.sync if h % 2 == 0 else nc.scalar).dma_start(
                    wo_t, wo_d[:, h * D:(h + 1) * D])
                wo_p.append(wo_t)

            qb = [qkp.tile([128, T], BF16, name=f"qb{h}") for h in range(HQ)]
            kb = qkp.tile([128, T], BF16, name="kb")
            vb = qkp.tile([128, T], BF16, name="vb")
            ao = [aop.tile([128, T], BF16, name=f"ao{h}") for h in range(HQ)]

            # ---------------- Phase A: QKV projections (feature-major Q/K)
            with tc.tile_pool(name="psA", bufs=1, space="PSUM") as psA:
                psq = [psA.tile([128, T], F32, name=f"psq{h}") for h in range(HQ)]
                psk = psA.tile([128, T], F32, name="psk")
                psv = psA.tile([128, T], F32, name="psv")
                def xk_of(k):
                    pi = 0
                    while k >= P0[pi + 1]:
                        pi += 1
                    lk = k - P0[pi]
                    return (xt_p[pi][:, lk * T:(lk + 1) * T],
                            wqk_p[pi][:, lk * WQK:(lk + 1) * WQK])

                for k in range(NK):
                    xk, wk_ = xk_of(k)
                    st, sp = (k == 0), (k == NK - 1)
                    for h in range(HQ):
                        nc.tensor.matmul(psq[h], wk_[:, h * HD:(h + 1) * HD],
                                         xk, start=st, stop=sp)
                    nc.tensor.matmul(psk, wk_[:, QF:QF + HD], xk,
                                     start=st, stop=sp)
                # V-pass m-outer: each column-slice of psv is one fully
                # closed accumulation group (one bank can hold only one
                # open group at a time).
                for m in range(B):
                    for k in range(NK):
                        xk, _ = xk_of(k)
                        nc.tensor.matmul(psv[:, m * S:(m + 1) * S],
                                         xk[:, m * S:(m + 1) * S],
                                         wv_t[:, k * HD:(k + 1) * HD],
                                         start=(k == 0), stop=(k == NK - 1))

                # ---- RoPE (feature-permuted: evens rows 0-63, odds 64-127)
                for src, dst in [(psk, kb)] + [(psq[h], qb[h]) for h in range(HQ)]:
                    e, o = src[0:64, :], src[64:128, :]
                    t1 = rtp.tile([64, T], F32, tag="t1")
                    t2 = rtp.tile([64, T], F32, tag="t2")
                    nc.vector.tensor_mul(t1, o, sinT)
                    nc.vector.tensor_mul(t2, e, cosT)
                    nc.vector.tensor_sub(dst[0:64, :], t2, t1)
                    t3 = rtp.tile([64, T], F32, tag="t1")
                    t4 = rtp.tile([64, T], F32, tag="t2")
                    nc.vector.tensor_mul(t3, o, cosT)
                    nc.vector.tensor_mul(t4, e, sinT)
                    nc.vector.tensor_add(dst[64:128, :], t4, t3)
                nc.vector.tensor_copy(vb, psv)

            # ---------------- Attention (per q head; layouts [j, i])
            with tc.tile_pool(name="psB", bufs=2, space="PSUM") as psB:
                for h in range(HQ):
                    psS = psB.tile([128, T], F32, tag="psS", name=f"psS{h}", bufs=2)
                    for m in range(B):
                        sl = slice(m * S, (m + 1) * S)
                        nc.tensor.matmul(psS[:, sl], kb[:, sl], qb[h][:, sl],
                                         start=True, stop=True)
                    au = ap_.tile([128, T], BF16, tag="au", name=f"au{h}")
                    nc.scalar.activation(au, psS, AF.Exp, scale=SCALE)
                    au2 = ap_.tile([128, T], BF16, tag="au2", name=f"au2{h}")
                    nc.vector.tensor_mul(au2, au, mkT)
                    pden = psB.tile([128, T], F32, tag="pden", name=f"pden{h}", bufs=1)
                    nc.tensor.matmul(pden, on128, au2, start=True, stop=True)
                    rec = ap_.tile([128, T], F32, tag="rec", name=f"rec{h}")
                    nc.vector.reciprocal(rec, pden)
                    psO = psB.tile([128, T], F32, tag="psO", name=f"psO{h}", bufs=1)
                    for m in range(B):
                        sl = slice(m * S, (m + 1) * S)
                        nc.tensor.matmul(psO[:, sl], vb[:, sl], au2[:, sl],
                                         start=True, stop=True)
                    nc.vector.tensor_mul(ao[h], psO, rec)

                # ---------------- Output projection
                NT = D // 512
                for m in range(B):
                    outm = op.tile([128, D], F32, tag="outm", name=f"outm{m}")
                    for n in range(NT):
                        pso = psB.tile([128, 512], F32, tag="pso", bufs=4,
                                       name=f"pso{m}_{n}")
                        for h in range(HQ):
                            nc.tensor.matmul(pso,
                                             ao[h][:, m * S:(m + 1) * S],
                                             wo_p[h][:, n * 512:(n + 1) * 512],
                                             start=(h == 0), stop=(h == HQ - 1))
                        if n % 2 == 0:
                            nc.vector.tensor_copy(outm[:, n * 512:(n + 1) * 512], pso)
                        else:
                            nc.scalar.copy(outm[:, n * 512:(n + 1) * 512], pso)
                    nc.sync.dma_start(out_d[m * S:(m + 1) * S, :], outm)

    nc.compile()
    return nc


_PERM = np.concatenate([np.arange(0, HD, 2), np.arange(1, HD, 2)])


def _prep_inputs(x, freqs_cos, freqs_sin, wq, bq, wk, bk, wv, bv, wo):
    bf = bfloat16
    xT = np.asarray(x, np.float32).reshape(T, D).T          # [D, T]
    xt_all = np.zeros((NK, 128, T), np.float32)
    xt_all[:NK - 1] = xT.reshape(NK - 1, 128, T)
    xt_all[NK - 1, 0, :] = 1.0
    xt_packed = np.ascontiguousarray(
        xt_all.transpose(1, 0, 2).reshape(128, NK * T)).astype(bf)
    cosT = np.ascontiguousarray(
        np.tile(np.asarray(freqs_cos, np.float32).T, (1, B)))
    sinT = np.ascontiguousarray(
        np.tile(np.asarray(freqs_sin, np.float32).T, (1, B)))
    mkT = np.ascontiguousarray(
        np.tile(np.triu(np.ones((S, S), np.float32)), (1, B))).astype(bf)
    on = np.ones((128, S), np.float32).astype(bf)
    wqf = np.asarray(wq, np.float32)
    bqf = np.asarray(bq, np.float32)
    wkf = np.asarray(wk, np.float32)
    bkf = np.asarray(bk, np.float32)
    wvf = np.asarray(wv, np.float32)
    bvf = np.asarray(bv, np.float32)
    wof = np.asarray(wo, np.float32)
    maps = []
    for c in range(NCORES):
        qs = slice(c * QF, (c + 1) * QF)
        ks = slice(c * HD, (c + 1) * HD)
        wq_c = wqf[:, qs].reshape(D, HQ, HD)[:, :, _PERM].reshape(D, QF)
        bq_c = bqf[qs].reshape(HQ, HD)[:, _PERM].reshape(QF)
        wk_c = wkf[:, ks][:, _PERM]
        bk_c = bkf[ks][_PERM]
        wqk = np.concatenate([wq_c, wk_c], axis=1)          # [D, 640]
        bqk = np.concatenate([bq_c, bk_c])
        wqk_all = np.zeros((NK, 128, WQK), np.float32)
        wqk_all[:NK - 1] = wqk.reshape(NK - 1, 128, WQK)
        wqk_all[NK - 1, 0, :] = bqk
        wqk_packed = np.ascontiguousarray(
            wqk_all.transpose(1, 0, 2).reshape(128, NK * WQK)).astype(bf)
        wv_all = np.zeros((NK, 128, HD), np.float32)
        wv_all[:NK - 1] = wvf[:, ks].reshape(NK - 1, 128, HD)
        wv_all[NK - 1, 0, :] = bvf[ks]
        wv_packed = np.ascontiguousarray(
            wv_all.transpose(1, 0, 2).reshape(128, NK * HD)).astype(bf)
        wo_packed = np.ascontiguousarray(
            wof[qs, :].reshape(HQ, 128, D).transpose(1, 0, 2)
            .reshape(128, HQ * D)).astype(bf)
        maps.append({
            "xt": xt_packed, "wqk": wqk_packed, "wv": wv_packed,
            "wo": wo_packed, "cosT": cosT, "sinT": sinT, "mkT": mkT, "on": on,
        })
    return maps


def kernel(x, start_pos, freqs_cos, freqs_sin, mask, cache_k, cache_v,
           wq, bq, wk, bk, wv, bv, wo, bo):
    from concourse.bass_utils import run_bass_kernel_spmd

    assert int(start_pos) == 0
    if "nc" not in _CACHE:
        _CACHE["nc"] = _build()
    nc = _CACHE["nc"]
    in_maps = _prep_inputs(np.asarray(x), np.asarray(freqs_cos),
                           np.asarray(freqs_sin), np.asarray(wq),
                           np.asarray(bq), np.asarray(wk), np.asarray(bk),
                           np.asarray(wv), np.asarray(bv), np.asarray(wo))
    res = run_bass_kernel_spmd(nc, in_maps, core_ids=list(range(NCORES)))
    acc = np.zeros((T, D), np.float64)
    for r in res.results:
        acc += r["out"].astype(np.float64)
    out = (acc + np.asarray(bo).astype(np.float64)).astype(np.float32)
    return out.reshape(B, S, D)


# revision 27
# speedup vs baseline: 114.0205x; 1.5996x over previous
"""GQA attention kernel for Trainium2, 8-core tensor-parallel (by heads).

Shapes (hardcoded from the problem spec):
  x:(4,128,4096) fp32, wq:(4096,4096), wk/wv:(4096,1024), wo:(4096,4096),
  32 q heads / 8 kv heads, head_dim 128, start_pos=0 (cache is overwritten).

Sharding: core c owns q heads [4c,4c+4) and kv head c; wq/wk/wv column-
sharded, wo row-sharded; each core computes a full (512,4096) partial of
the output projection; host sums the 8 partials and adds bo.

Device-side design (bf16 matmul path, fp32 accumulation):
  - Q/K projections computed feature-major (weights stationary, tokens
    moving) so attention needs no transposes.  Q/K features are permuted
    on the host to [evens, odds] within each head so RoPE is 6 DVE ops
    on contiguous partition halves.
  - Biases folded in as a 33rd contraction chunk (x row of ones, bias
    row in the weight pack).
  - All inputs repacked on the host into [128, k*cols] slabs so each
    array loads with O(1) large DMAs.
  - Softmax denominator via ones-matmul; 1/denom folded into the AV
    epilogue multiply.
"""
import sys
sys.path.insert(0, "/opt/trn_rl_repo")

import numpy as np
from ml_dtypes import bfloat16

B, S, D = 4, 128, 4096
H, KV, HD = 32, 8, 128
NCORES = 8
HQ = H // NCORES          # 4 q heads per core
T = B * S                 # 512 tokens
NK = D // 128 + 1         # 32 k-chunks + 1 bias chunk
QF = HQ * HD              # 512 q features per core
WQK = QF + HD             # 640 = q features + k features
SCALE = 1.0 / float(np.sqrt(HD))
PIECES = (2, 7, 8, 8, 8)  # k-chunks per phase-A DMA piece (small first piece
                          # so the PE starts as early as possible)

_CACHE = {}


def _build():
    import concourse.tile as tile
    from concourse import bacc, mybir

    F32, BF16 = mybir.dt.float32, mybir.dt.bfloat16
    AF = mybir.ActivationFunctionType

    nc = bacc.Bacc("TRN2", target_bir_lowering=False, debug=False,
                   enable_asserts=False, num_devices=NCORES)

    xt_d = nc.dram_tensor("xt", [128, NK * T], BF16, kind="ExternalInput").ap()
    wqk_d = nc.dram_tensor("wqk", [128, NK * WQK], BF16, kind="ExternalInput").ap()
    wv_d = nc.dram_tensor("wv", [128, NK * HD], BF16, kind="ExternalInput").ap()
    wo_d = nc.dram_tensor("wo", [128, HQ * D], BF16, kind="ExternalInput").ap()
    cosT_d = nc.dram_tensor("cosT", [128, T], BF16, kind="ExternalInput").ap()
    sinT_d = nc.dram_tensor("sinT", [128, T], BF16, kind="ExternalInput").ap()
    mkT_d = nc.dram_tensor("mkT", [128, T], BF16, kind="ExternalInput").ap()
    on_d = nc.dram_tensor("on", [128, S], BF16, kind="ExternalInput").ap()
    id_d = nc.dram_tensor("idm", [128, S], BF16, kind="ExternalInput").ap()
    out_d = nc.dram_tensor("out", [T, D], F32, kind="ExternalOutput").ap()

    P0 = [0]
    for p in PIECES:
        P0.append(P0[-1] + p)

    with tile.TileContext(nc) as tc:
        with tc.tile_pool(name="w", bufs=1) as wp, \
             tc.tile_pool(name="consts", bufs=1) as cp, \
             tc.tile_pool(name="qk", bufs=1) as qkp, \
             tc.tile_pool(name="rt", bufs=4) as rtp, \
             tc.tile_pool(name="attn", bufs=2) as ap_, \
             tc.tile_pool(name="aop", bufs=1) as aop, \
             tc.tile_pool(name="outp", bufs=6) as op:

            # ---- input DMAs: phase-A slabs first, split across both rings
            xt_p, wqk_p = [], []
            for i, npc in enumerate(PIECES):
                xt_t = wp.tile([128, npc * T], BF16, name=f"xtp{i}")
                wq_t = wp.tile([128, npc * WQK], BF16, name=f"wqkp{i}")
                e1, e2 = (nc.sync, nc.scalar) if i % 2 == 0 else (nc.scalar, nc.sync)
                e1.dma_start(xt_t, xt_d[:, P0[i] * T:P0[i + 1] * T])
                e2.dma_start(wq_t, wqk_d[:, P0[i] * WQK:P0[i + 1] * WQK])
                xt_p.append(xt_t)
                wqk_p.append(wq_t)
            wv_t = wp.tile([128, NK * HD], BF16, name="wv")
            nc.sync.dma_start(wv_t, wv_d)
            # cos/sin replicated into both partition halves so every
            # two-SBUF-operand DVE op has base-partition-aligned inputs.
            cosT = cp.tile([128, T], BF16)
            nc.scalar.dma_start(cosT, cosT_d)
            sinT = cp.tile([128, T], BF16)
            nc.scalar.dma_start(sinT, sinT_d)
            mkT = cp.tile([128, T], BF16)
            nc.sync.dma_start(mkT, mkT_d)
            on128 = cp.tile([128, S], BF16)
            nc.scalar.dma_start(on128, on_d)
            ident = cp.tile([128, S], BF16)
            nc.sync.dma_start(ident, id_d)
            wo_p = []
            for h in range(HQ):
                wo_t = wp.tile([128, D], BF16, name=f"wop{h}")
                (nc.sync if h % 2 == 0 else nc.scalar).dma_start(
                    wo_t, wo_d[:, h * D:(h + 1) * D])
                wo_p.append(wo_t)

            qb = [qkp.tile([128, T], BF16, name=f"qb{h}") for h in range(HQ)]
            kb = qkp.tile([128, T], BF16, name="kb")
            vb = qkp.tile([128, T], BF16, name="vb")
            ao = [aop.tile([128, T], BF16, name=f"ao{h}") for h in range(HQ)]

            # ---------------- Phase A: QKV projections (all feature-major)
            with tc.tile_pool(name="psA", bufs=1, space="PSUM") as psA:
                psq = [psA.tile([128, T], F32, name=f"psq{h}") for h in range(HQ)]
                psk = psA.tile([128, T], F32, name="psk")
                psv = psA.tile([128, T], F32, name="psv")
                def xk_of(k):
                    pi = 0
                    while k >= P0[pi + 1]:
                        pi += 1
                    lk = k - P0[pi]
                    return (xt_p[pi][:, lk * T:(lk + 1) * T],
                            wqk_p[pi][:, lk * WQK:(lk + 1) * WQK])

                # Tensor-major within each piece: K's accumulation closes
                # first in the last piece, so RoPE/attention overlap the
                # remaining Q/V matmuls instead of waiting for all of them.
                def lhs_of(t, k):
                    _, wk_ = xk_of(k)
                    if t == 0:
                        return wk_[:, QF:QF + HD]
                    if t <= HQ:
                        return wk_[:, (t - 1) * HD:t * HD]
                    return wv_t[:, k * HD:(k + 1) * HD]

                targets = [psk] + psq + [psv]
                for pi in range(len(PIECES)):
                    for t, tgt in enumerate(targets):
                        for k in range(P0[pi], P0[pi + 1]):
                            xk, _ = xk_of(k)
                            nc.tensor.matmul(tgt, lhs_of(t, k), xk,
                                             start=(k == 0), stop=(k == NK - 1))

                # ---- RoPE (feature-permuted: evens rows 0-63, odds 64-127)
                # Stage PSUM->SBUF bf16 on the ACT engine first; the six
                # rotate ops then run all-bf16-SBUF, hitting the DVE 4x mode.
                for src, dst in [(psk, kb)] + [(psq[h], qb[h]) for h in range(HQ)]:
                    sf = rtp.tile([128, T], BF16, tag="sf", bufs=3)
                    nc.scalar.copy(sf, src)
                    e, o = sf[0:64, :], sf[64:128, :]
                    t1 = rtp.tile([64, T], BF16, tag="t1")
                    t2 = rtp.tile([64, T], BF16, tag="t2")
                    nc.vector.tensor_mul(t1, o, sinT[64:128, :])
                    nc.vector.tensor_mul(t2, e, cosT[0:64, :])
                    nc.vector.tensor_sub(dst[0:64, :], t2, t1)
                    t3 = rtp.tile([64, T], BF16, tag="t1")
                    t4 = rtp.tile([64, T], BF16, tag="t2")
                    nc.vector.tensor_mul(t3, o, cosT[64:128, :])
                    nc.vector.tensor_mul(t4, e, sinT[0:64, :])
                    nc.vector.tensor_add(dst[64:128, :], t4, t3)
                # V was computed feature-major ([vf, tok]); transpose each
                # batch block through the PE to get token-major vb for AV.
                vfm = qkp.tile([128, T], BF16, name="vfm")
                nc.vector.tensor_copy(vfm, psv)
                for m in range(B):
                    pvT = psA.tile([128, S], BF16, tag="pvT", bufs=2,
                                   name=f"pvT{m}")
                    nc.tensor.transpose(pvT, vfm[:, m * S:(m + 1) * S], ident)
                    nc.vector.tensor_copy(vb[:, m * S:(m + 1) * S], pvT)

            # ---------------- Attention (per q head; layouts [j, i])
            with tc.tile_pool(name="psB", bufs=2, space="PSUM") as psB:
                for h in range(HQ):
                    psS = psB.tile([128, T], F32, tag="psS", name=f"psS{h}", bufs=2)
                    for m in range(B):
                        sl = slice(m * S, (m + 1) * S)
                        nc.tensor.matmul(psS[:, sl], kb[:, sl], qb[h][:, sl],
                                         start=True, stop=True)
                    au = ap_.tile([128, T], BF16, tag="au", name=f"au{h}")
                    nc.scalar.activation(au, psS, AF.Exp, scale=SCALE)
                    au2 = ap_.tile([128, T], BF16, tag="au2", name=f"au2{h}")
                    nc.vector.tensor_mul(au2, au, mkT)
                    pden = psB.tile([128, T], F32, tag="pden", name=f"pden{h}", bufs=1)
                    nc.tensor.matmul(pden, on128, au2, start=True, stop=True)
                    rec = ap_.tile([128, T], F32, tag="rec", name=f"rec{h}")
                    nc.vector.reciprocal(rec, pden)
                    psO = psB.tile([128, T], F32, tag="psO", name=f"psO{h}", bufs=1)
                    for m in range(B):
                        sl = slice(m * S, (m + 1) * S)
                        nc.tensor.matmul(psO[:, sl], vb[:, sl], au2[:, sl],
                                         start=True, stop=True)
                    nc.vector.tensor_mul(ao[h], psO, rec)

                # ---------------- Output projection
                # Out DMAs are issued per (m, n) tile on alternating rings so
                # the writeback fully overlaps the projection matmuls instead
                # of serializing into a tail on one ring.
                NT = D // 512
                for m in range(B):
                    for n in range(NT):
                        pso = psB.tile([128, 512], F32, tag="pso", bufs=4,
                                       name=f"pso{m}_{n}")
                        for h in range(HQ):
                            nc.tensor.matmul(pso,
                                             ao[h][:, m * S:(m + 1) * S],
                                             wo_p[h][:, n * 512:(n + 1) * 512],
                                             start=(h == 0), stop=(h == HQ - 1))
                        osb = op.tile([128, 512], F32, tag="osb",
                                      name=f"osb{m}_{n}")
                        if (m * NT + n) % 2 == 0:
                            nc.vector.tensor_copy(osb, pso)
                        else:
                            nc.scalar.copy(osb, pso)
                        eng = nc.sync if (m * NT + n) % 2 == 0 else nc.scalar
                        eng.dma_start(
                            out_d[m * S:(m + 1) * S, n * 512:(n + 1) * 512], osb)

    nc.compile()
    return nc


_PERM = np.concatenate([np.arange(0, HD, 2), np.arange(1, HD, 2)])


def _prep_inputs(x, freqs_cos, freqs_sin, wq, bq, wk, bk, wv, bv, wo):
    bf = bfloat16
    xT = np.asarray(x, np.float32).reshape(T, D).T          # [D, T]
    xt_all = np.zeros((NK, 128, T), np.float32)
    xt_all[:NK - 1] = xT.reshape(NK - 1, 128, T)
    xt_all[NK - 1, 0, :] = 1.0
    xt_packed = np.ascontiguousarray(
        xt_all.transpose(1, 0, 2).reshape(128, NK * T)).astype(bf)
    cosT = np.ascontiguousarray(
        np.tile(np.asarray(freqs_cos, np.float32).T, (2, B))).astype(bf)
    sinT = np.ascontiguousarray(
        np.tile(np.asarray(freqs_sin, np.float32).T, (2, B))).astype(bf)
    mkT = np.ascontiguousarray(
        np.tile(np.triu(np.ones((S, S), np.float32)), (1, B))).astype(bf)
    on = np.ones((128, S), np.float32).astype(bf)
    idm = np.eye(S, dtype=np.float32).astype(bf)
    wqf = np.asarray(wq, np.float32)
    bqf = np.asarray(bq, np.float32)
    wkf = np.asarray(wk, np.float32)
    bkf = np.asarray(bk, np.float32)
    wvf = np.asarray(wv, np.float32)
    bvf = np.asarray(bv, np.float32)
    wof = np.asarray(wo, np.float32)
    maps = []
    for c in range(NCORES):
        qs = slice(c * QF, (c + 1) * QF)
        ks = slice(c * HD, (c + 1) * HD)
        wq_c = wqf[:, qs].reshape(D, HQ, HD)[:, :, _PERM].reshape(D, QF)
        bq_c = bqf[qs].reshape(HQ, HD)[:, _PERM].reshape(QF)
        wk_c = wkf[:, ks][:, _PERM]
        bk_c = bkf[ks][_PERM]
        wqk = np.concatenate([wq_c, wk_c], axis=1)          # [D, 640]
        bqk = np.concatenate([bq_c, bk_c])
        wqk_all = np.zeros((NK, 128, WQK), np.float32)
        wqk_all[:NK - 1] = wqk.reshape(NK - 1, 128, WQK)
        wqk_all[NK - 1, 0, :] = bqk
        wqk_packed = np.ascontiguousarray(
            wqk_all.transpose(1, 0, 2).reshape(128, NK * WQK)).astype(bf)
        wv_all = np.zeros((NK, 128, HD), np.float32)
        wv_all[:NK - 1] = wvf[:, ks].reshape(NK - 1, 128, HD)
        wv_all[NK - 1, 0, :] = bvf[ks]
        wv_packed = np.ascontiguousarray(
            wv_all.transpose(1, 0, 2).reshape(128, NK * HD)).astype(bf)
        wo_packed = np.ascontiguousarray(
            wof[qs, :].reshape(HQ, 128, D).transpose(1, 0, 2)
            .reshape(128, HQ * D)).astype(bf)
        maps.append({
            "xt": xt_packed, "wqk": wqk_packed, "wv": wv_packed,
            "wo": wo_packed, "cosT": cosT, "sinT": sinT, "mkT": mkT, "on": on,
            "idm": idm,
        })
    return maps


def kernel(x, start_pos, freqs_cos, freqs_sin, mask, cache_k, cache_v,
           wq, bq, wk, bk, wv, bv, wo, bo):
    from concourse.bass_utils import run_bass_kernel_spmd

    assert int(start_pos) == 0
    if "nc" not in _CACHE:
        _CACHE["nc"] = _build()
    nc = _CACHE["nc"]
    in_maps = _prep_inputs(np.asarray(x), np.asarray(freqs_cos),
                           np.asarray(freqs_sin), np.asarray(wq),
                           np.asarray(bq), np.asarray(wk), np.asarray(bk),
                           np.asarray(wv), np.asarray(bv), np.asarray(wo))
    res = run_bass_kernel_spmd(nc, in_maps, core_ids=list(range(NCORES)))
    acc = np.zeros((T, D), np.float64)
    for r in res.results:
        acc += r["out"].astype(np.float64)
    out = (acc + np.asarray(bo).astype(np.float64)).astype(np.float32)
    return out.reshape(B, S, D)
